# revision 18
# baseline (speedup 1.0000x reference)
"""Trainium2 Bass kernel for nn_ContrastiveLoss (B=4096, D=512, 8 cores).

Strategy v5.2 (symmetric-triangle, fp8, dual-engine exp):
  - Host l2-normalizes [emb_i; emb_j] -> reps [8192, 512] (fp32), quantizes
    to fp8 e4m3, and packs per-core SBUF layouts.
  - The 8192x8192 similarity matrix is a 16x16 grid of 512x512 cells; only
    the 136 upper-triangle cells are computed (sim is symmetric).  One
    device tile [128 x 2048] = one cell: 4 row sub-slices (slots) x the
    cell's 512-column block.  136 cells = 17 per core, perfectly balanced.
  - Per-core tiles follow a fixed run pattern over x-blocks,
    [3,3,3,3,3,1,1]: each core loads only 7 x-blocks (1.84 MB) plus 17
    q-blocks (4.46 MB).
  - Per tile: fp8 DoubleRow matmuls (K=256/instr) -> 4-bank PSUM, then
    exp(5x) on ONE of TWO engines (split so neither is the bottleneck):
      * ACT tiles: ScalarE exp reading PSUM directly -> fp8.
      * SCH tiles: VectorE computes the fp8 BIT PATTERN of exp directly
        (Schraudolph): uint8 = round(sim * 40/ln2 + 56 - c).  The uint8
        codes ARE fp8e4m3 values of exp(5*sim) (max ~8% elem error, ~0
        mean bias) -- byte-compatible with the fp8 output buffer.
    Diagonal cells always go to ACT so the host's self-term subtraction
    (fp8(exp(diag))) matches the device bit-for-bit.
  - DMA: q0 issued first on the gpsimd queue while ALL x-blocks issue on
    the sync queue (in parallel), so tile 0 can start ~4us earlier than
    issuing everything serially on one queue.  Outputs follow on sync.
  - Host reduces: row sums of each cell feed its grid-row block, column
    sums of off-diagonal cells feed the transposed block, then the known
    fp8 diagonal term is subtracted and the host-computed fu scalar and
    positive-pair terms are added.

exp(5*sim) lies in [e^-1.5, e^5] ~ [0.2, 149]: inside fp8 e4m3 normal
range.  fp8 quantization of inputs+outputs yields ~5e-5 final loss error;
the Schraudolph tiles add ~0.3% per-element bias-free noise that averages
out in the 8192-term denominators (measured ~2e-4 on the final loss).
"""

import numpy as np

import concourse.bass as bass
import concourse.mybir as mybir
import concourse.tile as tile
from concourse import bacc

f32 = mybir.dt.float32
u8 = mybir.dt.uint8
fp8 = mybir.dt.float8e4
AF = mybir.ActivationFunctionType
ALU = mybir.AluOpType

P = 128
TEMP = 0.2
INV_T = 1.0 / TEMP  # 5.0
FP8_NP = mybir.dt.np(fp8)

B, D = 4096, 512
TWO_N = 2 * B           # 8192
KT = D // P             # 4 k-subtiles of 128
CHUNK = 2048            # ACT/psum tile free size (4 PSUM banks)
CELL = 512              # grid cell edge
G = TWO_N // CELL       # 16x16 grid
N_TILES = 17            # tiles (cells) per core
RUNS = [3, 3, 3, 3, 3, 1, 1]   # tiles per x-block slot
N_XB = len(RUNS)        # 7 x-block slots per core
QW = N_TILES * CELL     # 8704
XW = N_XB * CELL        # 3584
# x-slot of tile t under RUNS
XSLOT = [s for s, r in enumerate(RUNS) for _ in range(r)]

# Schraudolph constants: uint8 code = round(sim * S8 + B8) == fp8e4m3 bits
# of exp(INV_T * sim).  c=0.42 zeroes the mean relative error.
S8 = INV_T * 8.0 / np.log(2.0)   # 57.7078
B8 = 7.0 * 8.0 - 0.42            # 55.58
HALF = CHUNK // 2                # 1024: psum rotation granule (2 banks)


def _pieces():
    """Cut the 136 upper-triangle cells into 40 three-cell and 16 one-cell
    pieces, each piece within one grid column.  Ones are taken from the
    BACK of each column (highest rows), which makes 11 of the 16 ones
    diagonal cells -- enough to pin a diagonal cell at tile index 16 on
    every core (see _core_cells)."""
    threes, ones = [], []
    for j in range(G):
        cells = [(i, j) for i in range(j + 1)]
        rem = (j + 1) % 3
        for _ in range(rem):
            ones.append([cells.pop()])
        for k in range(0, len(cells), 3):
            threes.append(cells[k:k + 3])
    assert len(threes) == 40 and len(ones) == 16
    return threes, ones


# ones pairs (tile15, tile16): tile16 is ALWAYS a diagonal cell, so the
# SPMD-shared program can skip the below-diagonal sub-blocks of tile 16
# uniformly on every core (the host reconstructs them by symmetry).
ONES_PAIRS = [
    [(0, 1), (1, 1)], [(3, 4), (4, 4)], [(6, 7), (7, 7)],
    [(9, 10), (10, 10)], [(12, 13), (13, 13)],
    [(0, 0), (3, 3)], [(6, 6), (9, 9)], [(12, 12), (15, 15)],
]


def _core_cells():
    """Per-core list of 17 cells, ordered to match the RUNS pattern."""
    threes, ones = _pieces()
    used = {(c[0][0], c[0][1]) for pair in ONES_PAIRS for c in [pair]}
    flat_ones = [c for pair in ONES_PAIRS for c in pair]
    assert sorted(flat_ones) == sorted(c for o in ones for c in o)
    cores = []
    for c in range(8):
        pieces = threes[c * 5:(c + 1) * 5] + [[x] for x in ONES_PAIRS[c]]
        cores.append([cell for p in pieces for cell in p])
    for c in range(8):
        i, j = cores[c][16]
        assert i == j, f"tile16 must be diagonal on core {c}"
    return cores


CORE_CELLS = _core_cells()


def _sch_halves():
    """(tile, half) pairs whose exp runs on VectorE (Schraudolph).  The
    SPMD program is shared by all cores, so eligible tiles must be
    non-diagonal on EVERY core (ACT keeps diag cells bit-exact with the
    host self-term): {0,1,3,4,6,7,9,10,12,13,16}.  Assigning per-half
    (h0 on DVE, h1 on ACT for eligible tiles, plus two extra h1s) keeps
    both exp engines at ~1 half per tile with no queue buildup."""
    eligible = [t for t in range(N_TILES)
                if all(CORE_CELLS[c][t][0] != CORE_CELLS[c][t][1]
                       for c in range(8))]
    sch = {(t, 0) for t in eligible}
    mid = eligible[len(eligible) // 2:] + eligible[:len(eligible) // 2]
    for t in mid:
        if len(sch) >= 13:
            break
        sch.add((t, 1))
    return sch


SCH_HALVES = _sch_halves()


def build_nc():
    """SPMD program: tile t (= cell) computes, for slot g in 0..3,
    exp(5 * q[t*512+g*128 :][128] @ x[XSLOT[t]*512 :][512]) -> eout[:, t, g*512:]."""
    nc = bacc.Bacc("TRN2", target_bir_lowering=False, debug=False)

    qt_d = nc.dram_tensor("qt", [P, KT, QW], fp8, kind="ExternalInput")
    xt_d = nc.dram_tensor("xt", [P, KT, XW], fp8, kind="ExternalInput")
    out_d = nc.dram_tensor("eout", [P, N_TILES, CHUNK], fp8, kind="ExternalOutput")

    sch = SCH_HALVES
    perm = [15, 16, 14] + list(range(14))  # last tile (13) is eligible

    with tile.TileContext(nc) as tc:
        with (
            tc.tile_pool(name="qp", bufs=1) as qp,
            tc.tile_pool(name="xp", bufs=1) as xp,
            tc.tile_pool(name="wp", bufs=1) as wp,
            tc.tile_pool(name="scrp", bufs=1) as scrp,
            tc.tile_pool(name="psp", bufs=1, space="PSUM") as psp,
        ):
            qt_sb = qp.tile([P, KT, QW], fp8, tag="qt")
            xt_sb = xp.tile([P, KT, XW], fp8, tag="xt")
            scr_all = scrp.tile([P, N_TILES, CHUNK], fp8, tag="scr")
            # 4-deep PSUM rotation of [128, 1024] half-tiles (2 banks each):
            # depth 2 x 2048 cannot hide the exp+semaphore latency (the
            # bank for MM(t+2) frees ~600-900ns after PE needs it); depth 4
            # at half granularity gives ~1.1us of slack per slot.
            ps4 = [psp.tile([P, HALF], f32, tag=f"ps{i}", name=f"ps{i}")
                   for i in range(4)]

            # PE warm-up: dummy matmuls on a zeroed tile bridge the gap
            # until the first real data lands, holding the HAM clock-gate
            # at 8/8.
            warm = wp.tile([P, 2, 640], fp8, tag="warm")
            warm_scr = wp.tile([P, 1], fp8, tag="warm_scr")
            # q0 issues first-thing on the scalar queue (its only other
            # early work); memset on gpsimd so warmups start ~1.3us sooner
            t0 = perm[0]
            nc.scalar.dma_start(qt_sb[:, :, t0 * CELL:(t0 + 1) * CELL],
                                qt_d[:, :, t0 * CELL:(t0 + 1) * CELL])
            nc.gpsimd.memset(warm[:].bitcast(mybir.dt.uint32), 0)
            # load the exp ACT table during the DMA-wait head so the first
            # real ACT tile doesn't eat the ~2.7us table-load latency
            with nc.allow_low_precision(reason="warmup"):
                nc.scalar.activation(warm_scr[:], warm[:, 0, 0:1], AF.Exp,
                                     scale=INV_T)
            for _ in range(7):
                nc.tensor.matmul(
                    ps4[0][:, 0:512], warm[:, :, 0:P], warm[:, :, P:640],
                    start=True, stop=True,
                    perf_mode=mybir.MatmulPerfMode.DoubleRow)

            # Input DMAs: tile 0/1's q-blocks go on the otherwise-idle
            # scalar/vector queues (issued immediately after their engine
            # preambles, in parallel with x0 on sync), so tile 0 can start
            # ~3us earlier.  Remaining q-blocks stream on the gpsimd queue
            # in consumption order; x-blocks on sync ahead of the outputs.
            x_order = []
            for t in perm:
                s = XSLOT[t]
                if s not in x_order:
                    x_order.append(s)
            # sync: first-needed x-blocks and the 2nd tile's q; gpsimd: rest
            nc.sync.dma_start(
                xt_sb[:, :, x_order[0] * CELL:(x_order[0] + 1) * CELL],
                xt_d[:, :, x_order[0] * CELL:(x_order[0] + 1) * CELL])
            t1 = perm[1]
            nc.sync.dma_start(qt_sb[:, :, t1 * CELL:(t1 + 1) * CELL],
                              qt_d[:, :, t1 * CELL:(t1 + 1) * CELL])
            for s in x_order[1:]:
                nc.sync.dma_start(
                    xt_sb[:, :, s * CELL:(s + 1) * CELL],
                    xt_d[:, :, s * CELL:(s + 1) * CELL])
            for t in perm[2:]:
                nc.gpsimd.dma_start(
                    qt_sb[:, :, t * CELL:(t + 1) * CELL],
                    qt_d[:, :, t * CELL:(t + 1) * CELL])

            for pos, t in enumerate(perm):
                x0 = XSLOT[t] * CELL
                for h in range(2):
                    ps = ps4[(2 * pos + h) % 4]
                    for kt in range(2):
                        for gg in range(2):
                            g = 2 * h + gg
                            q0 = t * CELL + g * P
                            # tile 16 is diagonal on every core: skip the
                            # below-diagonal sub-blocks (cols < g*128)
                            c0 = g * P if t == N_TILES - 1 else 0
                            nc.tensor.matmul(
                                ps[:, gg * 512 + c0:(gg + 1) * 512],
                                qt_sb[:, 2 * kt:2 * kt + 2, q0:q0 + P],
                                xt_sb[:, 2 * kt:2 * kt + 2, x0 + c0:x0 + CELL],
                                start=(kt == 0), stop=(kt == 1),
                                perf_mode=mybir.MatmulPerfMode.DoubleRow,
                            )
                    scr = scr_all[:, t, h * HALF:(h + 1) * HALF]
                    if (t, h) in sch:
                        # VectorE Schraudolph: fp8 bit pattern of exp(5*sim)
                        nc.vector.tensor_scalar(
                            scr.bitcast(u8), ps[:], S8, B8, ALU.mult, ALU.add)
                    else:
                        with nc.allow_low_precision(reason="fp8 exp output is the design"):
                            nc.scalar.activation(scr, ps[:], AF.Exp, scale=INV_T)
                if pos < N_TILES - 1:
                    nc.sync.dma_start(out_d[:, t, :], scr_all[:, t, :])
                else:
                    # split the last tile's output so the tail only waits on
                    # the second half's transfer
                    nc.sync.dma_start(out_d[:, t, 0:HALF],
                                      scr_all[:, t, 0:HALF])
                    nc.sync.dma_start(out_d[:, t, HALF:CHUNK],
                                      scr_all[:, t, HALF:CHUNK])

    nc.finalize()
    return nc


def _l2n(x):
    n = np.sqrt(np.sum(x.astype(np.float32) ** 2, axis=1, keepdims=True))
    return x / np.maximum(n, 1e-12)


def _pack(z8):
    """[rows, 512] fp8 -> [128, 4, rows] SBUF layout: out[p,k,n] = z8[n, k*128+p]."""
    return np.ascontiguousarray(z8.T.reshape(KT, P, -1).transpose(1, 0, 2))


def prepare(emb_i, emb_j, emb_k):
    z_i = _l2n(emb_i)
    z_j = _l2n(emb_j)
    z_k = _l2n(emb_k)
    reps = np.concatenate([z_i, z_j], axis=0).astype(np.float32)  # [8192, 512]
    z8 = reps.astype(FP8_NP)
    z8f = z8.astype(np.float32)

    packed = _pack(z8)  # [128, 4, 8192]
    blk = [packed[:, :, b * CELL:(b + 1) * CELL] for b in range(G)]
    in_maps = []
    for c in range(8):
        cells = CORE_CELLS[c]
        qt = np.concatenate([blk[i] for i, _ in cells], axis=2)
        xb = []
        for t, (_, j) in enumerate(cells):
            if t == 0 or XSLOT[t] != XSLOT[t - 1]:
                xb.append(blk[j])
        xt = np.concatenate(xb, axis=2)
        in_maps.append({"qt": np.ascontiguousarray(qt),
                        "xt": np.ascontiguousarray(xt)})

    pos = np.sum(z_i.astype(np.float64) * z_j.astype(np.float64), axis=1)
    sim_ik = np.sum(z_k.astype(np.float64) * z_i.astype(np.float64), axis=1)
    denom_fu = 2.0 * np.sum(np.exp(sim_ik * INV_T))
    diag = np.sum(z8f.astype(np.float64) * z8f.astype(np.float64), axis=1)
    self_term = np.exp(diag * INV_T).astype(FP8_NP).astype(np.float64)
    ctx = {"pos2": np.concatenate([pos, pos]), "denom_fu": denom_fu,
           "self_term": self_term}
    return in_maps, ctx


def assemble(results, ctx):
    """Row sums + symmetric column sums of the fp8 exp cells -> loss.

    Tile 16 is a diagonal cell whose below-diagonal sub-blocks were skipped
    on-device (stale bytes in eout): mask them and reconstruct by symmetry
    from the computed upper sub-blocks."""
    S = np.zeros(TWO_N, dtype=np.float64)
    for c, r in enumerate(results):
        e = np.asarray(r["eout"]).astype(np.float32)   # [128, 17, 2048]
        e4 = e.reshape(P, N_TILES, 4, CELL)
        e16 = e4[:, 16].astype(np.float64).copy()      # [128, 4, 512]
        for g in range(1, 4):
            e16[:, g, :g * P] = 0.0
        e4 = e4.copy()
        e4[:, 16] = e16
        rsum = e4.sum(axis=3, dtype=np.float64)        # [128, 17, 4]
        csum = e4.sum(axis=0, dtype=np.float64)        # [17, 4, 512]
        for t, (i, j) in enumerate(CORE_CELLS[c]):
            for g in range(4):
                S[i * CELL + g * P:i * CELL + (g + 1) * P] += rsum[:, t, g]
            if i != j:
                S[j * CELL:(j + 1) * CELL] += csum[t].sum(axis=0)
        # symmetric lower part of tile 16: row group g, cols < g*128
        i16 = CORE_CELLS[c][16][0]
        cg16 = e16.sum(axis=0)                          # [4, 512]
        for g in range(1, 4):
            lower = cg16[:g, g * P:(g + 1) * P].sum(axis=0)   # [128]
            S[i16 * CELL + g * P:i16 * CELL + (g + 1) * P] += lower
    denom = S - ctx["self_term"] + ctx["denom_fu"]
    loss = np.mean(np.log(denom) - INV_T * ctx["pos2"])
    return np.asarray(np.float32(loss))


_NC_CACHE = {}


def _get_nc():
    if "nc" not in _NC_CACHE:
        _NC_CACHE["nc"] = build_nc()
    return _NC_CACHE["nc"]


def kernel(emb_i, emb_j, emb_k):
    from concourse.bass_utils import run_bass_kernel_spmd

    emb_i = np.asarray(emb_i, dtype=np.float32)
    emb_j = np.asarray(emb_j, dtype=np.float32)
    emb_k = np.asarray(emb_k, dtype=np.float32)
    in_maps, ctx = prepare(emb_i, emb_j, emb_k)
    nc = _get_nc()
    res = run_bass_kernel_spmd(nc, in_maps, list(range(8))).results
    return assemble(res, ctx)
